# revision 3
# baseline (speedup 1.0000x reference)
"""CeNN layer (nn_CeNNLayer) Trainium2 Bass kernel.

Problem: x [16,64,128,128] f32; per image:
    ic    = conv3x3(x, B_w) + B_b + Z
    s0    = conv3x3(x, rescale_w) + rescale_b
    s_{k+1} = s_k + 0.1*(-s_k + conv3x3(nonlin(s_k), A_w) + A_b + ic),  10 iters
    out   = nonlin(s_10)
with nonlin(v) = max(min(v, 1+a(v-1)), -1+a(min(v,1+a(v-1))+1)), a=0.01.

Sharding: data-parallel over batch, 2 images per NeuronCore on 8 cores.

Per-core layout ("quadrant pixel-split"): every per-pixel tensor is
[128 partitions, ...] where partition p<64 holds channel p of image rows 0-63
(half A) and partition p>=64 holds channel p-64 of rows 64-127 (half B).
The 3x3 conv runs as 9 accumulating K=64 fp16 matmuls per 4-row tile on the
PE-array quadrants (0,0) (half A) and (64,64) (half B) concurrently, into one
[128,512] fp32 psum bank. Cross-half halo rows are exchanged by SBUF-SBUF DMA.

nonlin is computed as  z := nonlin(s)+1 = Lrelu(2 - Lrelu(1 - s)),  slope a,
on the Scalar engine; z is stored padded with pad value 1.0 (z=nl+1 makes the
reference's zero padding a constant), so padding folds into a per-channel bias
vb = 0.1*(B_b+Z+A_b) - 0.1*sum(A_w).  State update s' = 0.9 s + psum + IC is
two fused Vector-engine ops.  z is double-buffered across iterations (stencil
hazard) and the per-iteration tile order rotates by 2 so producers of the next
iteration's first tiles always land early.
"""
import numpy as np

import concourse.bacc as bacc
import concourse.mybir as mybir
import concourse.tile as tile
from concourse.bass_utils import run_bass_kernel_spmd

F32 = mybir.dt.float32
F16 = mybir.dt.float16

ALPHA = 0.01
N_CORES = 8
NIMG = 2            # images per core (batch 16 / 8 cores)
ROWS = 66           # buffer rows: 1 pad/halo + 64 data + 1 pad/halo
PITCH = 130         # 1 pad col + 128 data cols + 1 pad col
NT = 16             # 4-row tiles per half-image
NPIX = 64 * 128     # pixels per half-image
ITERS = 10
TAPS = [(dy, dx) for dy in (-1, 0, 1) for dx in (-1, 0, 1)]

_NC_CACHE = None


def build_nc():
    nc = bacc.Bacc(None, target_bir_lowering=False)

    xp_d = nc.dram_tensor("xp", [128, NIMG, ROWS, PITCH], F16, kind="ExternalInput")
    wt_d = nc.dram_tensor("wt", [128, 27 * 64], F16, kind="ExternalInput")
    bias_d = nc.dram_tensor("bias", [128, 2], F32, kind="ExternalInput")
    yo_d = nc.dram_tensor("yo", [128, NIMG, NPIX], F32, kind="ExternalOutput")

    with tile.TileContext(nc) as tc:
        with (
            tc.tile_pool(name="main", bufs=1) as main,
            tc.tile_pool(name="scr", bufs=6) as scr,
            tc.tile_pool(name="psc", bufs=6, space="PSUM") as psc,
            tc.tile_pool(name="psi", bufs=2, space="PSUM") as psi,
        ):
            xt = main.tile([128, NIMG, ROWS, PITCH], F16)
            zA = main.tile([128, ROWS, PITCH], F16)
            zB = main.tile([128, ROWS, PITCH], F16)
            zbufs = [zA, zB]
            stA = main.tile([128, NPIX], F32)
            stB = main.tile([128, NPIX], F32)
            st_bufs = [stA, stB]
            ict = main.tile([128, NPIX], F32)
            wt = main.tile([128, 27 * 64], F16)
            bt = main.tile([128, 2], F32)
            b1 = main.tile([128, 1], F32)
            b2 = main.tile([128, 1], F32)

            nc.sync.dma_start(wt[:], wt_d[:])
            nc.sync.dma_start(bt[:], bias_d[:])
            nc.sync.dma_start(xt[:, 0:1, 0:21, :], xp_d[:, 0:1, 0:21, :])
            nc.sync.dma_start(xt[:, 0:1, 21:ROWS, :], xp_d[:, 0:1, 21:ROWS, :])
            nc.sync.dma_start(xt[:, 1:2, :, :], xp_d[:, 1:2, :, :])
            nc.gpsimd.memset(b1[:], 1.0)
            nc.gpsimd.memset(b2[:], 2.0)
            nc.gpsimd.memset(zA[:], 1.0)
            nc.gpsimd.memset(zB[:], 1.0)

            LR = mybir.ActivationFunctionType.Lrelu
            ID = mybir.ActivationFunctionType.Identity

            def conv9(psum, wblk, rhs_fn, t):
                # accumulate 9 taps into psum for 4-row tile t, both quadrants
                r0 = 1 + 4 * t
                for j, (dy, dx) in enumerate(TAPS):
                    for pb in (0, 64):
                        nc.tensor.matmul(
                            psum[pb:pb + 64, :],
                            wt[pb:pb + 64, (wblk + j) * 64:(wblk + j + 1) * 64],
                            rhs_fn(pb, r0 + dy, 1 + dx),
                            start=(j == 0),
                            stop=(j == 8),
                            tile_position=(pb, pb),
                            skip_group_check=True,
                        )

            for img in range(NIMG):
                st = st_bufs[img]
                def xrhs(pb, r, c, img=img):
                    return xt[pb:pb + 64, img:img + 1, r:r + 4, c:c + 128]

                def zrhs_for(zt):
                    def zrhs(pb, r, c):
                        return zt[pb:pb + 64, r:r + 4, c:c + 128]
                    return zrhs

                # setup: state0 = conv(x, rescale)+rescale_b; IC = 0.1conv(x,B)+vb
                g0 = img * (ITERS + 1)
                ss = (2 * g0) % NT
                for t in [(ss + i) % NT for i in range(NT)]:
                    off = 512 * t
                    pr = psc.tile([128, 512], F32, tag="conv")
                    pi = psi.tile([128, 512], F32, tag="ic")
                    conv9(pr, 0, xrhs, t)
                    conv9(pi, 9, xrhs, t)
                    nc.scalar.activation(st[:, off:off + 512], pr[:], ID,
                                         bias=bt[:, 0:1], scale=1.0)
                    nc.vector.tensor_scalar(ict[:, off:off + 512], pi[:],
                                            bt[:, 1:2], None,
                                            mybir.AluOpType.add)
                    # z0 = Lrelu(2 - Lrelu(1 - state0)) = nonlin(state0) + 1
                    u = scr.tile([128, 512], F32, tag="u")
                    nc.scalar.activation(u[:], st[:, off:off + 512], LR,
                                         bias=b1[:], scale=-1.0, alpha=ALPHA)
                    r0 = 1 + 4 * t
                    nc.scalar.activation(zbufs[0][:, r0:r0 + 4, 1:129], u[:], LR,
                                         bias=b2[:], scale=-1.0, alpha=ALPHA)
                    if t == NT - 1:
                        # half B top halo <- half A last data row
                        nc.sync.dma_start(zbufs[0][64:128, 0, :], zbufs[0][0:64, 64, :])
                    if t == 0:
                        # half A bottom halo <- half B first data row
                        nc.sync.dma_start(zbufs[0][0:64, 65, :], zbufs[0][64:128, 1, :])

                for it in range(1, ITERS + 1):
                    last = it == ITERS
                    zprev = zbufs[(it + 1) % 2]
                    znext = zbufs[it % 2]
                    s = (2 * (g0 + it)) % NT
                    for t in [(s + i) % NT for i in range(NT)]:
                        off = 512 * t
                        p = psc.tile([128, 512], F32, tag="conv")
                        conv9(p, 18, zrhs_for(zprev), t)
                        tmp = scr.tile([128, 512], F32, tag="tmp")
                        # tmp = 0.9*state + psum ; state = tmp + IC
                        nc.vector.scalar_tensor_tensor(
                            out=tmp[:], in0=st[:, off:off + 512], scalar=0.9,
                            in1=p[:], op0=mybir.AluOpType.mult,
                            op1=mybir.AluOpType.add)
                        nc.vector.tensor_tensor(
                            st[:, off:off + 512], tmp[:], ict[:, off:off + 512],
                            mybir.AluOpType.add)
                        u = scr.tile([128, 512], F32, tag="u")
                        nc.scalar.activation(u[:], st[:, off:off + 512], LR,
                                             bias=b1[:], scale=-1.0, alpha=ALPHA)
                        if not last:
                            r0 = 1 + 4 * t
                            nc.scalar.activation(znext[:, r0:r0 + 4, 1:129], u[:], LR,
                                                 bias=b2[:], scale=-1.0, alpha=ALPHA)
                            if t == NT - 1:
                                nc.sync.dma_start(znext[64:128, 0, :], znext[0:64, 64, :])
                            if t == 0:
                                nc.sync.dma_start(znext[0:64, 65, :], znext[64:128, 1, :])
                        else:
                            zf = scr.tile([128, 512], F32, tag="zf")
                            nc.scalar.activation(zf[:], u[:], LR,
                                                 bias=b2[:], scale=-1.0, alpha=ALPHA)
                            nc.vector.tensor_scalar(
                                st[:, off:off + 512], zf[:], -1.0, None,
                                mybir.AluOpType.add)

                for oc in range(4):
                    nc.sync.dma_start(yo_d[:, img, oc * 2048:(oc + 1) * 2048],
                                      st[:, oc * 2048:(oc + 1) * 2048])

    nc.compile()
    return nc


def pack_inputs(x, rescale_w, rescale_b, A_w, A_b, B_w, B_b, Z, n_cores=N_CORES):
    """Host-side prep: pad/split x per core, build fp16 lhsT tap blocks, biases."""
    x = np.asarray(x, dtype=np.float32)

    def lhsT_blocks(w):  # [co,ci,3,3] -> [64, 9*64] fp16, cols = tap-major, co
        out = np.empty((64, 9 * 64), dtype=np.float16)
        for j, (dy, dx) in enumerate(TAPS):
            out[:, j * 64:(j + 1) * 64] = w[:, :, dy + 1, dx + 1].T.astype(np.float16)
        return out

    wt = np.zeros((128, 27 * 64), dtype=np.float16)
    half = np.concatenate(
        [lhsT_blocks(np.asarray(rescale_w)),
         lhsT_blocks(0.1 * np.asarray(B_w)),
         lhsT_blocks(0.1 * np.asarray(A_w))], axis=1)
    wt[0:64] = half
    wt[64:128] = half

    # vb = 0.1(B_b+Z+A_b) - CA, CA = sum of the fp16 A-taps actually used
    A16 = wt[0:64, 18 * 64:27 * 64].astype(np.float32).reshape(64, 9, 64)
    CA = A16.sum(axis=(0, 1))
    vb = (0.1 * (np.asarray(B_b) + np.asarray(Z) + np.asarray(A_b)) - CA).astype(np.float32)
    bias = np.zeros((128, 2), dtype=np.float32)
    bias[0:64, 0] = rescale_b
    bias[64:128, 0] = rescale_b
    bias[0:64, 1] = vb
    bias[64:128, 1] = vb

    in_maps = []
    for c in range(n_cores):
        xp = np.zeros((128, NIMG, ROWS, PITCH), dtype=np.float16)
        for i in range(NIMG):
            g = x[c * NIMG + i]  # [64, 128, 128]
            xp[0:64, i, 1:65, 1:129] = g[:, 0:64, :]
            xp[0:64, i, 65, 1:129] = g[:, 64, :]
            xp[64:128, i, 1:65, 1:129] = g[:, 64:128, :]
            xp[64:128, i, 0, 1:129] = g[:, 63, :]
        in_maps.append({"xp": xp, "wt": wt, "bias": bias})
    return in_maps


def unpack_outputs(results, n_cores=N_CORES):
    out = np.empty((n_cores * NIMG, 64, 128, 128), dtype=np.float32)
    for c in range(n_cores):
        yo = results[c]["yo"]  # [128, NIMG, NPIX]
        for i in range(NIMG):
            out[c * NIMG + i, :, 0:64, :] = yo[0:64, i].reshape(64, 64, 128)
            out[c * NIMG + i, :, 64:128, :] = yo[64:128, i].reshape(64, 64, 128)
    return out


def kernel(x, rescale_w, rescale_b, A_w, A_b, B_w, B_b, Z, **_):
    global _NC_CACHE
    if _NC_CACHE is None:
        _NC_CACHE = build_nc()
    in_maps = pack_inputs(x, rescale_w, rescale_b, A_w, A_b, B_w, B_b, Z)
    res = run_bass_kernel_spmd(_NC_CACHE, in_maps, list(range(N_CORES)))
    return unpack_outputs(res.results)


# revision 4
# speedup vs baseline: 1.0104x; 1.0104x over previous
"""CeNN layer (nn_CeNNLayer) Trainium2 Bass kernel.

Problem: x [16,64,128,128] f32; per image:
    ic    = conv3x3(x, B_w) + B_b + Z
    s0    = conv3x3(x, rescale_w) + rescale_b
    s_{k+1} = s_k + 0.1*(-s_k + conv3x3(nonlin(s_k), A_w) + A_b + ic),  10 iters
    out   = nonlin(s_10)
with nonlin(v) = max(min(v, 1+a(v-1)), -1+a(min(v,1+a(v-1))+1)), a=0.01.

Sharding: data-parallel over batch, 2 images per NeuronCore on 8 cores.

Per-core layout ("quadrant pixel-split"): every per-pixel tensor is
[128 partitions, ...] where partition p<64 holds channel p of image rows 0-63
(half A) and partition p>=64 holds channel p-64 of rows 64-127 (half B).
The 3x3 conv runs as 9 accumulating K=64 fp16 matmuls per 4-row tile on the
PE-array quadrants (0,0) (half A) and (64,64) (half B) concurrently, into one
[128,512] fp32 psum bank. Cross-half halo rows are exchanged by SBUF-SBUF DMA.

nonlin is computed as  z := nonlin(s)+1 = Lrelu(2 - Lrelu(1 - s)),  slope a,
on the Scalar engine; z is stored padded with pad value 1.0 (z=nl+1 makes the
reference's zero padding a constant), so padding folds into a per-channel bias
vb = 0.1*(B_b+Z+A_b) - 0.1*sum(A_w).  State update s' = 0.9 s + psum + IC is
two fused Vector-engine ops.  z is double-buffered across iterations (stencil
hazard) and the per-iteration tile order rotates by 2 so producers of the next
iteration's first tiles always land early.
"""
import numpy as np

import concourse.bacc as bacc
import concourse.mybir as mybir
import concourse.tile as tile
from concourse.bass_utils import run_bass_kernel_spmd

F32 = mybir.dt.float32
F16 = mybir.dt.float16

ALPHA = 0.01
N_CORES = 8
NIMG = 2            # images per core (batch 16 / 8 cores)
ROWS = 66           # buffer rows: 1 pad/halo + 64 data + 1 pad/halo
PITCH = 130         # 1 pad col + 128 data cols + 1 pad col
NT = 16             # 4-row tiles per half-image
NPIX = 64 * 128     # pixels per half-image
ITERS = 10
TAPS = [(dy, dx) for dy in (-1, 0, 1) for dx in (-1, 0, 1)]

_NC_CACHE = None


def build_nc():
    nc = bacc.Bacc(None, target_bir_lowering=False)

    xp_d = nc.dram_tensor("xp", [128, NIMG, ROWS, PITCH], F16, kind="ExternalInput")
    wt_d = nc.dram_tensor("wt", [128, 27 * 64], F16, kind="ExternalInput")
    bias_d = nc.dram_tensor("bias", [128, 2], F32, kind="ExternalInput")
    yo_d = nc.dram_tensor("yo", [128, NIMG, NPIX], F32, kind="ExternalOutput")

    with tile.TileContext(nc) as tc:
        with (
            tc.tile_pool(name="main", bufs=1) as main,
            tc.tile_pool(name="scr", bufs=6) as scr,
            tc.tile_pool(name="psc", bufs=5, space="PSUM") as psc,
            tc.tile_pool(name="psi", bufs=3, space="PSUM") as psi,
        ):
            xt = main.tile([128, NIMG, ROWS, PITCH], F16)
            zA = main.tile([128, ROWS, PITCH], F16)
            zB = main.tile([128, ROWS, PITCH], F16)
            zbufs = [zA, zB]
            stA = main.tile([128, NPIX], F32)
            stB = main.tile([128, NPIX], F32)
            st_bufs = [stA, stB]
            ict = main.tile([128, NPIX], F32)
            wt = main.tile([128, 27 * 64], F16)
            bt = main.tile([128, 2], F32)
            b1 = main.tile([128, 1], F32)
            b2 = main.tile([128, 1], F32)

            nc.sync.dma_start(wt[:], wt_d[:])
            nc.sync.dma_start(bt[:], bias_d[:])
            nc.sync.dma_start(xt[:, 0:1, 0:21, :], xp_d[:, 0:1, 0:21, :])
            nc.sync.dma_start(xt[:, 0:1, 21:ROWS, :], xp_d[:, 0:1, 21:ROWS, :])
            nc.sync.dma_start(xt[:, 1:2, :, :], xp_d[:, 1:2, :, :])
            nc.gpsimd.memset(b1[:], 1.0)
            nc.gpsimd.memset(b2[:], 2.0)
            nc.gpsimd.memset(zA[:], 1.0)
            nc.gpsimd.memset(zB[:], 1.0)

            LR = mybir.ActivationFunctionType.Lrelu
            ID = mybir.ActivationFunctionType.Identity

            def conv9(psum, wblk, rhs_fn, t):
                # accumulate 9 taps into psum for 4-row tile t, both quadrants
                r0 = 1 + 4 * t
                for j, (dy, dx) in enumerate(TAPS):
                    for pb in (0, 64):
                        nc.tensor.matmul(
                            psum[pb:pb + 64, :],
                            wt[pb:pb + 64, (wblk + j) * 64:(wblk + j + 1) * 64],
                            rhs_fn(pb, r0 + dy, 1 + dx),
                            start=(j == 0),
                            stop=(j == 8),
                            tile_position=(pb, pb),
                            skip_group_check=True,
                        )

            for img in range(NIMG):
                st = st_bufs[img]
                def xrhs(pb, r, c, img=img):
                    return xt[pb:pb + 64, img:img + 1, r:r + 4, c:c + 128]

                def zrhs_for(zt):
                    def zrhs(pb, r, c):
                        return zt[pb:pb + 64, r:r + 4, c:c + 128]
                    return zrhs

                # setup: state0 = conv(x, rescale)+rescale_b; IC = 0.1conv(x,B)+vb
                g0 = img * (ITERS + 1)
                ss = (2 * g0) % NT
                for t in [(ss + i) % NT for i in range(NT)]:
                    off = 512 * t
                    pr = psc.tile([128, 512], F32, tag="conv")
                    pi = psi.tile([128, 512], F32, tag="ic")
                    conv9(pr, 0, xrhs, t)
                    conv9(pi, 9, xrhs, t)
                    nc.scalar.activation(st[:, off:off + 512], pr[:], ID,
                                         bias=bt[:, 0:1], scale=1.0)
                    nc.vector.tensor_scalar(ict[:, off:off + 512], pi[:],
                                            bt[:, 1:2], None,
                                            mybir.AluOpType.add)
                    # z0 = Lrelu(2 - Lrelu(1 - state0)) = nonlin(state0) + 1
                    u = scr.tile([128, 512], F32, tag="u")
                    nc.scalar.activation(u[:], st[:, off:off + 512], LR,
                                         bias=b1[:], scale=-1.0, alpha=ALPHA)
                    r0 = 1 + 4 * t
                    nc.scalar.activation(zbufs[0][:, r0:r0 + 4, 1:129], u[:], LR,
                                         bias=b2[:], scale=-1.0, alpha=ALPHA)
                    if t == NT - 1:
                        # half B top halo <- half A last data row
                        nc.sync.dma_start(zbufs[0][64:128, 0, :], zbufs[0][0:64, 64, :])
                    if t == 0:
                        # half A bottom halo <- half B first data row
                        nc.sync.dma_start(zbufs[0][0:64, 65, :], zbufs[0][64:128, 1, :])

                for it in range(1, ITERS + 1):
                    last = it == ITERS
                    zprev = zbufs[(it + 1) % 2]
                    znext = zbufs[it % 2]
                    s = (2 * (g0 + it)) % NT
                    for t in [(s + i) % NT for i in range(NT)]:
                        off = 512 * t
                        p = psc.tile([128, 512], F32, tag="conv")
                        conv9(p, 18, zrhs_for(zprev), t)
                        tmp = scr.tile([128, 512], F32, tag="tmp")
                        # tmp = 0.9*state + psum ; state = tmp + IC
                        nc.vector.scalar_tensor_tensor(
                            out=tmp[:], in0=st[:, off:off + 512], scalar=0.9,
                            in1=p[:], op0=mybir.AluOpType.mult,
                            op1=mybir.AluOpType.add)
                        nc.vector.tensor_tensor(
                            st[:, off:off + 512], tmp[:], ict[:, off:off + 512],
                            mybir.AluOpType.add)
                        u = scr.tile([128, 512], F32, tag="u")
                        nc.scalar.activation(u[:], st[:, off:off + 512], LR,
                                             bias=b1[:], scale=-1.0, alpha=ALPHA)
                        if not last:
                            r0 = 1 + 4 * t
                            nc.scalar.activation(znext[:, r0:r0 + 4, 1:129], u[:], LR,
                                                 bias=b2[:], scale=-1.0, alpha=ALPHA)
                            if t == NT - 1:
                                nc.sync.dma_start(znext[64:128, 0, :], znext[0:64, 64, :])
                            if t == 0:
                                nc.sync.dma_start(znext[0:64, 65, :], znext[64:128, 1, :])
                        else:
                            zf = scr.tile([128, 512], F32, tag="zf")
                            nc.scalar.activation(zf[:], u[:], LR,
                                                 bias=b2[:], scale=-1.0, alpha=ALPHA)
                            nc.vector.tensor_scalar(
                                st[:, off:off + 512], zf[:], -1.0, None,
                                mybir.AluOpType.add)

                for oc in range(4):
                    nc.sync.dma_start(yo_d[:, img, oc * 2048:(oc + 1) * 2048],
                                      st[:, oc * 2048:(oc + 1) * 2048])

    nc.compile()
    return nc


def pack_inputs(x, rescale_w, rescale_b, A_w, A_b, B_w, B_b, Z, n_cores=N_CORES):
    """Host-side prep: pad/split x per core, build fp16 lhsT tap blocks, biases."""
    x = np.asarray(x, dtype=np.float32)

    def lhsT_blocks(w):  # [co,ci,3,3] -> [64, 9*64] fp16, cols = tap-major, co
        out = np.empty((64, 9 * 64), dtype=np.float16)
        for j, (dy, dx) in enumerate(TAPS):
            out[:, j * 64:(j + 1) * 64] = w[:, :, dy + 1, dx + 1].T.astype(np.float16)
        return out

    wt = np.zeros((128, 27 * 64), dtype=np.float16)
    half = np.concatenate(
        [lhsT_blocks(np.asarray(rescale_w)),
         lhsT_blocks(0.1 * np.asarray(B_w)),
         lhsT_blocks(0.1 * np.asarray(A_w))], axis=1)
    wt[0:64] = half
    wt[64:128] = half

    # vb = 0.1(B_b+Z+A_b) - CA, CA = sum of the fp16 A-taps actually used
    A16 = wt[0:64, 18 * 64:27 * 64].astype(np.float32).reshape(64, 9, 64)
    CA = A16.sum(axis=(0, 1))
    vb = (0.1 * (np.asarray(B_b) + np.asarray(Z) + np.asarray(A_b)) - CA).astype(np.float32)
    bias = np.zeros((128, 2), dtype=np.float32)
    bias[0:64, 0] = rescale_b
    bias[64:128, 0] = rescale_b
    bias[0:64, 1] = vb
    bias[64:128, 1] = vb

    in_maps = []
    for c in range(n_cores):
        xp = np.zeros((128, NIMG, ROWS, PITCH), dtype=np.float16)
        for i in range(NIMG):
            g = x[c * NIMG + i]  # [64, 128, 128]
            xp[0:64, i, 1:65, 1:129] = g[:, 0:64, :]
            xp[0:64, i, 65, 1:129] = g[:, 64, :]
            xp[64:128, i, 1:65, 1:129] = g[:, 64:128, :]
            xp[64:128, i, 0, 1:129] = g[:, 63, :]
        in_maps.append({"xp": xp, "wt": wt, "bias": bias})
    return in_maps


def unpack_outputs(results, n_cores=N_CORES):
    out = np.empty((n_cores * NIMG, 64, 128, 128), dtype=np.float32)
    for c in range(n_cores):
        yo = results[c]["yo"]  # [128, NIMG, NPIX]
        for i in range(NIMG):
            out[c * NIMG + i, :, 0:64, :] = yo[0:64, i].reshape(64, 64, 128)
            out[c * NIMG + i, :, 64:128, :] = yo[64:128, i].reshape(64, 64, 128)
    return out


def kernel(x, rescale_w, rescale_b, A_w, A_b, B_w, B_b, Z, **_):
    global _NC_CACHE
    if _NC_CACHE is None:
        _NC_CACHE = build_nc()
    in_maps = pack_inputs(x, rescale_w, rescale_b, A_w, A_b, B_w, B_b, Z)
    res = run_bass_kernel_spmd(_NC_CACHE, in_maps, list(range(N_CORES)))
    return unpack_outputs(res.results)
